# revision 1
# baseline (speedup 1.0000x reference)
"""Trainium2 Bass kernel for nn_HausdorffDistance_28406913696124.

Math (reference):
    px = (prob_map[0].ravel() >= 0.5)                 # [N], N = 100*100
    py = (gt_map.ravel()   >= 0.5)                    # [N]
    D[i,j] = euclid dist between grid points i, j     # [N, N] constant!
    loss   = mean_i | px_i * mean_j D[i,j] - (D @ py)_i / N |

Key structure: D depends only on (|r_i-r_j|, |c_i-c_j|) with r=i//100,
c=i%100.  So:
  * rowmean_i = mean_j D[i,j] is a pure constant -> precomputed on host.
  * (D @ py) is a 2D correlation of the 100x100 binary mask PY with the
    199x199 kernel sqrt(dr^2+dc^2).  Factor it through the distance table
    Q[u,v] = sqrt(u^2+v^2), u,v in [0,100):

        term2sum[r,c] = sum_d ( H_d[r-d, c] + H_d[r+d, c] )   (d=0 once)
        H_d  = PY @ T_d          T_d[b,c] = Q[d, |b-c|]   (Toeplitz)

    On the PE the +-d row shifts are folded into the stationary operand:
    for the 13 d's owned by a core (d = 13k + j, j = 0..12) the stationary
    C_j[b, r] = PYT_plus[b, r-j] + PYT_minus[b, r+j], where PYT_plus/minus
    are the transposed binary mask pre-shifted by +-13k (shift baked into
    the per-core input data, so the SPMD program only uses j = 0..12 as
    compile-time AP offsets).  One strided DVE add builds all 13 C_j from
    zero-padded tiles; 13 accumulating matmuls then produce this core's
    partial term2sum [100,100] directly in PSUM.

Sharding: 100 d-values split across 8 cores (13/core, zero padded), an
on-device AllReduce sums the partial term2sum maps, then every core
computes the identical final scalar.
"""

import sys

import numpy as np

sys.path.insert(0, "/opt/trn_rl_repo")

H = 100
N = H * H
NCORES = 8
DSH = 13   # d-values per core (8*13 = 104 >= 100, rest zero-padded)
PADW = 13  # zero pad on each side of the transposed-mask tiles
CHUNK = 500  # free-dim chunk (matmul <= 512 fp32 PSUM bank)


def _host_constants():
    """Geometry-only constant tables (input independent)."""
    idx = np.arange(H)
    absdiff = np.abs(idx[:, None] - idx[None, :])  # [100,100] |b-c|
    # fp32-exact integer squares -> correctly rounded fp32 sqrt: matches the
    # reference's gram-matrix + sqrt exactly.
    q32 = np.sqrt((idx[:, None] ** 2 + idx[None, :] ** 2).astype(np.float32))

    # rowsum[r,c] = sum_j D[i,j] (i = r*100+c), accumulated in float64.
    # (N * rowmean -- the 1/N^2 is folded into the final scalar scale.)
    cnt = np.zeros((H, H))  # cnt[r,u] = #{a : |r-a| = u}
    np.add.at(cnt, (idx[:, None], absdiff), 1.0)
    # negated so the device can fold "- px*rowsumN" into the 9-way
    # gather-sum (see _build_module).
    rowsumN = (-(cnt @ q32.astype(np.float64) @ cnt.T)).astype(np.float32)

    q16 = q32.astype(np.float16)
    t_slices = []
    for k in range(NCORES):
        t_k = np.zeros((H, DSH * H), dtype=np.float16)
        for j in range(DSH):
            d = k * DSH + j
            if d >= H:
                continue
            blk = q16[d, absdiff]
            if d == 0:
                # d=0 appears in both the +j and -j branch of the combined
                # stationary; halve once so it is counted once.
                blk = (blk.astype(np.float32) * 0.5).astype(np.float16)
            t_k[:, j * H:(j + 1) * H] = blk
        t_slices.append(t_k)
    return rowsumN, t_slices


def _build_module(with_collective=True):
    import concourse.bacc as bacc
    import concourse.mybir as mybir
    import concourse.tile as tile

    f32 = mybir.dt.float32
    f16 = mybir.dt.float16

    nc = bacc.Bacc(
        "TRN2",
        target_bir_lowering=False,
        debug=False,
        enable_asserts=False,
        num_devices=NCORES,
    )

    # gtpack = gtT_plus | gtT_minus ; rmprob = rowsumN | prob  ([100,100] f32)
    gtpack_d = nc.dram_tensor("gtpack", [H, 2 * H], f32, kind="ExternalInput")
    rmprob_d = nc.dram_tensor("rmprob", [H, 2 * H], f32, kind="ExternalInput")
    tsl_d = nc.dram_tensor("t_slice", [H, DSH * H], f16, kind="ExternalInput")
    out_d = nc.dram_tensor("out", [1, 1], f32, kind="ExternalOutput")

    PW = H + 2 * PADW  # padded width of the transposed-mask tiles

    with tile.TileContext(nc) as tc:
        with (
            tc.tile_pool(name="sb", bufs=1) as sb,
            tc.tile_pool(name="ps_acc", bufs=1, space="PSUM") as ps_acc,
            tc.tile_pool(name="ps_fin", bufs=1, space="PSUM") as ps_fin,
            tc.tile_pool(name="dram", bufs=1, space="DRAM") as dram,
        ):
            # ---- loads (gt/rm on ACT ring, T on SP ring; a single
            # InstDMACopy is split across all 16 SDMA engines on HW) ------
            gtpack_sb = sb.tile([H, 2 * H], f32)
            nc.scalar.dma_start(gtpack_sb[:], gtpack_d[:])
            gtp_sb = gtpack_sb[:, 0:H]
            gtm_sb = gtpack_sb[:, H:2 * H]
            rmprob_sb = sb.tile([H, 2 * H], f32)
            nc.scalar.dma_start(rmprob_sb[:], rmprob_d[:])
            rm_sb = rmprob_sb[:, 0:H]
            prob_sb = rmprob_sb[:, H:2 * H]

            tsl_sb = sb.tile([H, DSH * H], f16)
            nc.sync.dma_start(tsl_sb[:], tsl_d[:])

            # ---- binarize the pre-shifted transposed masks -------------
            pytp = sb.tile([H, PW], f16)  # PYT_plus, zero padded
            pytm = sb.tile([H, PW], f16)  # PYT_minus, zero padded
            nc.vector.memset(pytp[:], 0.0)
            nc.vector.memset(pytm[:], 0.0)
            nc.vector.tensor_scalar(
                pytp[:, PADW:PADW + H], gtp_sb, 0.5, None, mybir.AluOpType.is_ge
            )
            nc.vector.tensor_scalar(
                pytm[:, PADW:PADW + H], gtm_sb, 0.5, None, mybir.AluOpType.is_ge
            )

            # ---- combined stationary: C_j[b, m] = pytp[b, PADW-j+m]
            #                                     + pytm[b, PADW+j+m] ----
            comb = sb.tile([H, DSH * H], f16)
            for j in range(DSH):
                nc.vector.tensor_add(
                    comb[:, j * H:(j + 1) * H],
                    pytp[:, PADW - j:PADW - j + H],
                    pytm[:, PADW + j:PADW + j + H],
                )

            # ---- 13 accumulating matmuls -> partial term2sum in PSUM ---
            acc_ps = ps_acc.tile([H, H], f32)
            for j in range(DSH):
                nc.tensor.matmul(
                    acc_ps[:],
                    comb[:, j * H:(j + 1) * H],
                    tsl_sb[:, j * H:(j + 1) * H],
                    start=(j == 0),
                    stop=(j == DSH - 1),
                )
            # ---- AllGather the 8 partial maps, sum them on-device ------
            # (AG floor ~5us vs AR ~10us on 8 cores; the 8-way sum is one
            # strided DVE reduce over a [100, 100, 8] view.)
            part2 = sb.tile([H, H], f32)
            nc.vector.tensor_copy(part2[:], acc_ps[:])
            cc_in = dram.tile([H, H], f32)
            cc_out = dram.tile([NCORES * H, H], f32, addr_space="Shared")
            nc.sync.dma_start(cc_in[:], part2[:])
            if with_collective:
                nc.gpsimd.collective_compute(
                    "AllGather",
                    mybir.AluOpType.bypass,
                    replica_groups=[list(range(NCORES))],
                    ins=[cc_in[:].opt()],
                    outs=[cc_out[:].opt()],
                )
                gath_src = cc_out[:]
            else:
                # timing-model variant (no collectives in sim): fake the
                # gather with a single same-sized DRAM read.
                gath_src = cc_out[:]
            # gath slices g=0..7: the gathered partial maps; slice 8:
            # t1n = px * (-rowsumN).  One strided 9-way reduce then gives
            # diff = term2sum - px*rowsumN directly.
            gath = sb.tile([H, (NCORES + 1) * H], f32)
            nc.vector.scalar_tensor_tensor(
                gath[:, NCORES * H:(NCORES + 1) * H],
                prob_sb,
                0.5,
                rm_sb,
                op0=mybir.AluOpType.is_ge,
                op1=mybir.AluOpType.mult,
            )
            # DRAM [g*H + p, c] -> SBUF [p, g*H + c]
            nc.scalar.dma_start(
                gath[:, 0:NCORES * H].rearrange("p (g c) -> p g c", g=NCORES),
                gath_src.rearrange("(g p) c -> p g c", g=NCORES),
            )
            diff = sb.tile([H, H], f32)
            nc.vector.tensor_reduce(
                diff[:],
                gath[:].rearrange("p (g c) -> p c g", g=NCORES + 1),
                axis=mybir.AxisListType.X,
                op=mybir.AluOpType.add,
            )
            rowsums = sb.tile([H, 1], f32)
            nc.vector.tensor_reduce(
                rowsums[:],
                diff[:],
                axis=mybir.AxisListType.X,
                op=mybir.AluOpType.add,
                apply_absolute_value=True,
            )
            ones_sb = sb.tile([H, 1], f32)
            nc.vector.memset(ones_sb[:], 1.0)
            fin_ps = ps_fin.tile([1, 1], f32)
            nc.tensor.matmul(fin_ps[:], rowsums[:], ones_sb[:])
            out_sb = sb.tile([1, 1], f32)
            nc.vector.tensor_scalar_mul(out_sb[:], fin_ps[:], 1.0 / (N * N))
            nc.sync.dma_start(out_d[:], out_sb[:])

    nc.compile()
    return nc


_STATE = {}


def _get_state():
    if not _STATE:
        rowsumN, t_slices = _host_constants()
        _STATE["consts"] = (rowsumN, t_slices)
        _STATE["nc"] = _build_module()
    return _STATE


def _in_maps(prob_map, gt_map):
    st = _get_state()
    rowsumN, t_slices = st["consts"]
    prob = np.asarray(prob_map, dtype=np.float32).reshape(H, H)
    gt = np.asarray(gt_map, dtype=np.float32).reshape(H, H)
    gtT = np.ascontiguousarray(gt.T)

    rmprob = np.ascontiguousarray(np.concatenate([rowsumN, prob], axis=1))
    in_maps = []
    for k in range(NCORES):
        dk = k * DSH
        gtp = np.zeros((H, H), dtype=np.float32)
        gtm = np.zeros((H, H), dtype=np.float32)
        gtp[:, dk:] = gtT[:, :H - dk]
        gtm[:, :H - dk] = gtT[:, dk:]
        gtpack = np.ascontiguousarray(np.concatenate([gtp, gtm], axis=1))
        in_maps.append(
            {"gtpack": gtpack, "rmprob": rmprob, "t_slice": t_slices[k]}
        )
    return in_maps


def _run(prob_map, gt_map, trace=False, **spmd_kwargs):
    from concourse import bass_utils

    st = _get_state()
    in_maps = _in_maps(prob_map, gt_map)
    res = bass_utils.run_bass_kernel_spmd(
        st["nc"], in_maps, core_ids=list(range(NCORES)), trace=trace,
        **spmd_kwargs,
    )
    value = np.float32(res.results[0]["out"][0, 0])
    return value, res


def kernel(prob_map, gt_map):
    value, _ = _run(prob_map, gt_map, trace=False)
    return np.asarray(value, dtype=np.float32)



# revision 4
# speedup vs baseline: 1.8434x; 1.8434x over previous
"""Trainium2 Bass kernel for nn_HausdorffDistance_28406913696124.

Math (reference):
    px = (prob_map[0].ravel() >= 0.5)                 # [N], N = 100*100
    py = (gt_map.ravel()   >= 0.5)                    # [N]
    D[i,j] = euclid dist between grid points i, j     # [N, N] constant!
    loss   = mean_i | px_i * mean_j D[i,j] - (D @ py)_i / N |

Key structure: with pixels i = (r, c), D depends only on the lag pair
(|r-a|, |c-b|), so

    term2[r,c] = (D @ py)[r,c] = sum_{a,b} PY[a,b] * K(|r-a|, |c-b|),
    K(u,v) = sqrt(u^2 + v^2).

K is numerically LOW-RANK: its eigenvalues on the [0,100)^2 lag grid decay
as 8046, -962, -68, -12, -2.9, ...  A rank-4 symmetric eigen-expansion
K ~ sum_m lam_m w_m w_m^T makes term2 separable:

    term2 = sum_m TA_m^T @ PY @ TB_m,   TA_m/TB_m 100x100 symmetric
    Toeplitz tables toep(w_m)*sqrt|lam_m| (sign folded into TA_m).

End-to-end elementwise error of the rank-4 fp16-table pipeline vs the
exact oracle is < 4e-4 (tolerance 2e-2).

Device program per core (ALL work on device, ~15 instructions):
    pytbin = (gtT >= 0.5)                    # DVE, fp16 [100,100]
    X      = PY @ TBstack                    # 1 matmul  [100 x 100 x 400]
    acc    = -px .* rowsum*1e-8              # DVE writes PSUM (term1, exact
                                             #  host-precomputed rowsums)
    acc   += sum_m TA_m^T @ X_m              # 4 accumulating matmuls
    out    = sum |acc|                       # one gpsimd XYZWC abs-reduce
The 1/N^2 is folded into the constant tables (1e-4 per factor side).

Distribution: the whole problem is ~160KB of constants + 2 matmul chains;
a cross-core reduction would cost more in collective latency (~5-15us
floor) than 8-way parallelism saves, so the kernel is replicated on all
8 cores (each computes the identical full scalar, no collectives) and
core 0's output is returned.
"""

import sys

import numpy as np

sys.path.insert(0, "/opt/trn_rl_repo")

H = 100
N = H * H
NCORES = 8
R = 4       # separable rank (eigen tail at rank 4: ~1e-4 relative)
S1 = 1e-4   # per-side scale; S1*S1 = 1/N^2 folds the final mean


def _host_constants():
    """Geometry-only constant tables (input independent)."""
    idx = np.arange(H)
    absdiff = np.abs(idx[:, None] - idx[None, :])  # [100,100] |lag|
    # fp32-exact integer squares -> correctly rounded fp32 sqrt matches the
    # reference's gram-matrix + sqrt construction of D.
    q32 = np.sqrt((idx[:, None] ** 2 + idx[None, :] ** 2).astype(np.float32))

    # Exact per-pixel rowsums of D, accumulated in float64 (term1 path).
    cnt = np.zeros((H, H))  # cnt[r,u] = #{a : |r-a| = u}
    np.add.at(cnt, (idx[:, None], absdiff), 1.0)
    rowsum = cnt @ q32.astype(np.float64) @ cnt.T  # [100,100], ~5.7e5
    rowsum_neg_scaled = (-rowsum * (S1 * S1)).astype(np.float32)

    # Rank-R symmetric eigen-factorization of the lag kernel.
    lam, w = np.linalg.eigh(q32.astype(np.float64))
    order = np.argsort(-np.abs(lam))
    lam, w = lam[order], w[:, order]
    tb = np.zeros((H, R * H), dtype=np.float16)
    ta = np.zeros((H, R * H), dtype=np.float16)
    for m in range(R):
        toep = w[:, m][absdiff] * (np.sqrt(abs(lam[m])) * S1)
        tb[:, m * H:(m + 1) * H] = toep.astype(np.float16)
        ta[:, m * H:(m + 1) * H] = (np.sign(lam[m]) * toep).astype(np.float16)
    return rowsum_neg_scaled, ta, tb


def _build_module():
    import concourse.bacc as bacc
    import concourse.bass_isa as bass_isa
    import concourse.mybir as mybir
    import concourse.tile as tile

    f32 = mybir.dt.float32
    f16 = mybir.dt.float16

    nc = bacc.Bacc(
        "TRN2",
        target_bir_lowering=False,
        debug=False,
        enable_asserts=False,
        num_devices=NCORES,
    )

    # pk = gtT | prob | rowsum_neg_scaled   ([100, 300] f32)
    pk_d = nc.dram_tensor("pk", [H, 3 * H], f32, kind="ExternalInput")
    tb_d = nc.dram_tensor("tb", [H, R * H], f16, kind="ExternalInput")
    ta_d = nc.dram_tensor("ta", [H, R * H], f16, kind="ExternalInput")
    out_d = nc.dram_tensor("out", [1, 1], f32, kind="ExternalOutput")

    with tile.TileContext(nc) as tc:
        with (
            tc.tile_pool(name="sb", bufs=1) as sb,
            tc.tile_pool(name="ps_x", bufs=1, space="PSUM") as ps_x,
            tc.tile_pool(name="ps_a", bufs=1, space="PSUM") as ps_a,
        ):
            # ---- loads: pk on the Pool/SWDGE queue (desc-gen starts at
            # t~0.1us, no HWDGE contention), tables on the SP/HWDGE queue;
            # the two transfers run on independent desc-gen paths. --------
            pk_sb = sb.tile([H, 3 * H], f32)
            nc.gpsimd.dma_start(pk_sb[:], pk_d[:])
            tb_sb = sb.tile([H, R * H], f16)
            nc.sync.dma_start(tb_sb[:], tb_d[:])
            ta_sb = sb.tile([H, R * H], f16)
            nc.sync.dma_start(ta_sb[:], ta_d[:])

            gtT_sb = pk_sb[:, 0:H]
            prob_sb = pk_sb[:, H:2 * H]
            rs_sb = pk_sb[:, 2 * H:3 * H]

            # ---- binarize transposed gt mask (mm1 stationary) -----------
            pytbin = sb.tile([H, H], f16)
            nc.vector.tensor_scalar(
                pytbin[:], gtT_sb, 0.5, None, mybir.AluOpType.is_ge
            )

            # ---- term1 pre-loaded into the mm2 accumulator: ------------
            #      acc = (prob >= 0.5) * (-rowsum * 1e-8)
            acc_ps = ps_a.tile([H, H], f32)
            nc.vector.scalar_tensor_tensor(
                acc_ps[:],
                prob_sb,
                0.5,
                rs_sb,
                op0=mybir.AluOpType.is_ge,
                op1=mybir.AluOpType.mult,
            )

            # ---- X = PY @ TBstack  (one 400-wide matmul) ---------------
            x_ps = ps_x.tile([H, R * H], f32)
            nc.tensor.matmul(x_ps[:], pytbin[:], tb_sb[:], start=True, stop=True)
            x_sb = sb.tile([H, R * H], f16)
            nc.vector.tensor_copy(x_sb[:], x_ps[:])

            # ---- acc += sum_m TA_m^T @ X_m  (4 accumulating matmuls) ---
            for m in range(R):
                nc.tensor.matmul(
                    acc_ps[:],
                    ta_sb[:, m * H:(m + 1) * H],
                    x_sb[:, m * H:(m + 1) * H],
                    start=False,
                    stop=(m == R - 1),
                )

            # ---- scalar: abs-reduce rows on DVE (PSUM -> SBUF [100,1]),
            # then an in-partition all-reduce on gpsimd, write out -------
            absrow = sb.tile([H, 1], f32)
            nc.vector.tensor_reduce(
                absrow[:],
                acc_ps[:],
                axis=mybir.AxisListType.X,
                op=mybir.AluOpType.add,
                apply_absolute_value=True,
            )
            red = sb.tile([H, 1], f32)
            nc.gpsimd.partition_all_reduce(
                red[:], absrow[:], channels=H, reduce_op=bass_isa.ReduceOp.add
            )
            nc.sync.dma_start(out_d[:], red[0:1, :])

    nc.compile()
    return nc


_STATE = {}


def _get_state():
    if not _STATE:
        _STATE["consts"] = _host_constants()
        _STATE["nc"] = _build_module()
    return _STATE


def _in_maps(prob_map, gt_map):
    st = _get_state()
    rowsum_neg_scaled, ta, tb = st["consts"]
    prob = np.asarray(prob_map, dtype=np.float32).reshape(H, H)
    gt = np.asarray(gt_map, dtype=np.float32).reshape(H, H)
    pk = np.ascontiguousarray(
        np.concatenate([gt.T, prob, rowsum_neg_scaled], axis=1)
    )
    in_map = {"pk": pk, "tb": tb, "ta": ta}
    return [in_map] * NCORES


def _run(prob_map, gt_map, trace=False, **spmd_kwargs):
    from concourse import bass_utils

    st = _get_state()
    in_maps = _in_maps(prob_map, gt_map)
    res = bass_utils.run_bass_kernel_spmd(
        st["nc"], in_maps, core_ids=list(range(NCORES)), trace=trace,
        **spmd_kwargs,
    )
    value = np.float32(res.results[0]["out"][0, 0])
    return value, res


def kernel(prob_map, gt_map):
    value, _ = _run(prob_map, gt_map, trace=False)
    return np.asarray(value, dtype=np.float32)


# revision 7
# speedup vs baseline: 1.9288x; 1.0463x over previous
"""Trainium2 Bass kernel for nn_HausdorffDistance_28406913696124.

Math (reference):
    px = (prob_map[0].ravel() >= 0.5)                 # [N], N = 100*100
    py = (gt_map.ravel()   >= 0.5)                    # [N]
    D[i,j] = euclid dist between grid points i, j     # [N, N] constant!
    loss   = mean_i | px_i * mean_j D[i,j] - (D @ py)_i / N |

Key structure: with pixels i = (r, c), D depends only on the lag pair
(|r-a|, |c-b|), so

    term2[r,c] = (D @ py)[r,c] = sum_{a,b} PY[a,b] * K(|r-a|, |c-b|),
    K(u,v) = sqrt(u^2 + v^2).

K is numerically LOW-RANK: its eigenvalues on the [0,100)^2 lag grid decay
as 8046, -962, -68, -12, -2.9, ...  A rank-4 symmetric eigen-expansion
K ~ sum_m lam_m w_m w_m^T makes term2 separable:

    term2 = sum_m TA_m^T @ PY @ TB_m,   TA_m/TB_m 100x100 symmetric
    Toeplitz tables toep(w_m)*sqrt|lam_m| (sign folded into TA_m).

End-to-end elementwise error of the rank-4 fp16-table pipeline vs the
exact oracle is < 4e-4 (tolerance 2e-2).

Device program per core (ALL work on device, ~15 instructions):
    pytbin = (gtT >= 0.5)                    # DVE, fp16 [100,100]
    X      = PY @ TBstack                    # 1 matmul  [100 x 100 x 400]
    acc    = -px .* rowsum*1e-8              # DVE writes PSUM (term1, exact
                                             #  host-precomputed rowsums)
    acc   += sum_m TA_m^T @ X_m              # 4 accumulating matmuls
    out    = sum |acc|                       # one gpsimd XYZWC abs-reduce
The 1/N^2 is folded into the constant tables (1e-4 per factor side).

Distribution: the whole problem is ~160KB of constants + 2 matmul chains;
a cross-core reduction would cost more in collective latency (~5-15us
floor) than 8-way parallelism saves, so the kernel is replicated on all
8 cores (each computes the identical full scalar, no collectives) and
core 0's output is returned.
"""

import sys

import numpy as np

sys.path.insert(0, "/opt/trn_rl_repo")

H = 100
N = H * H
NCORES = 8
R = 4       # separable rank (eigen tail at rank 4: ~1e-4 relative)
S1 = 1e-4   # per-side scale; S1*S1 = 1/N^2 folds the final mean


def _host_constants():
    """Geometry-only constant tables (input independent)."""
    idx = np.arange(H)
    absdiff = np.abs(idx[:, None] - idx[None, :])  # [100,100] |lag|
    # fp32-exact integer squares -> correctly rounded fp32 sqrt matches the
    # reference's gram-matrix + sqrt construction of D.
    q32 = np.sqrt((idx[:, None] ** 2 + idx[None, :] ** 2).astype(np.float32))

    # Exact per-pixel rowsums of D, accumulated in float64 (term1 path).
    cnt = np.zeros((H, H))  # cnt[r,u] = #{a : |r-a| = u}
    np.add.at(cnt, (idx[:, None], absdiff), 1.0)
    rowsum = cnt @ q32.astype(np.float64) @ cnt.T  # [100,100], ~5.7e5
    rowsum_neg_scaled = (-rowsum * (S1 * S1)).astype(np.float32)

    # Rank-R symmetric eigen-factorization of the lag kernel.
    lam, w = np.linalg.eigh(q32.astype(np.float64))
    order = np.argsort(-np.abs(lam))
    lam, w = lam[order], w[:, order]
    tb = np.zeros((H, R * H), dtype=np.float16)
    ta = np.zeros((H, R * H), dtype=np.float16)
    for m in range(R):
        toep = w[:, m][absdiff] * (np.sqrt(abs(lam[m])) * S1)
        tb[:, m * H:(m + 1) * H] = toep.astype(np.float16)
        ta[:, m * H:(m + 1) * H] = (np.sign(lam[m]) * toep).astype(np.float16)
    return rowsum_neg_scaled, ta, tb


def _build_module():
    import concourse.bacc as bacc
    import concourse.bass as bass
    import concourse.bass_isa as bass_isa
    import concourse.mybir as mybir
    import concourse.tile as tile

    f32 = mybir.dt.float32
    f16 = mybir.dt.float16

    # Bass.__init__ registers four const-AP memsets on the gpsimd/Pool
    # queue; they are unused here but delay the Pool-queue input DMA's
    # SWDGE descriptor generation by ~0.6us.  Route them to the (idle at
    # t=0, and long idle afterwards) DVE queue instead.
    orig_memset = bass.BassGpSimd.memset

    def _memset_on_dve(self, ap, constant):
        return self.bass.vector.memset(ap, constant)

    bass.BassGpSimd.memset = _memset_on_dve
    try:
        nc = bacc.Bacc(
            "TRN2",
            target_bir_lowering=False,
            debug=False,
            enable_asserts=False,
            num_devices=NCORES,
        )
    finally:
        bass.BassGpSimd.memset = orig_memset

    # pk = gtT | prob | rowsum_neg_scaled   ([100, 300] f32)
    pk_d = nc.dram_tensor("pk", [H, 3 * H], f32, kind="ExternalInput")
    tb_d = nc.dram_tensor("tb", [H, R * H], f16, kind="ExternalInput")
    ta_d = nc.dram_tensor("ta", [H, R * H], f16, kind="ExternalInput")
    out_d = nc.dram_tensor("out", [1, 1], f32, kind="ExternalOutput")

    with tile.TileContext(nc) as tc:
        with (
            tc.tile_pool(name="sb", bufs=1) as sb,
            tc.tile_pool(name="ps", bufs=1, space="PSUM") as ps,
        ):
            # ---- loads: pk on the Pool/SWDGE queue (desc-gen starts at
            # t~0.1us, no HWDGE contention), tables on the SP/HWDGE queue;
            # the two transfers run on independent desc-gen paths. --------
            pk_sb = sb.tile([H, 3 * H], f32)
            nc.gpsimd.dma_start(pk_sb[:], pk_d[:])
            tb_sb = sb.tile([H, R * H], f16)
            nc.sync.dma_start(tb_sb[:], tb_d[:])
            ta_sb = sb.tile([H, R * H], f16)
            nc.sync.dma_start(ta_sb[:], ta_d[:])

            gtT_sb = pk_sb[:, 0:H]
            prob_sb = pk_sb[:, H:2 * H]
            rs_sb = pk_sb[:, 2 * H:3 * H]

            # ---- binarize transposed gt mask (mm1 stationary) -----------
            pytbin = sb.tile([H, H], f16)
            nc.vector.tensor_scalar(
                pytbin[:], gtT_sb, 0.5, None, mybir.AluOpType.is_ge
            )

            # ---- term1 pre-loaded into the mm2 accumulator: ------------
            #      acc = (prob >= 0.5) * (-rowsum * 1e-8)
            acc_ps = ps.tile([H, H], f32)
            nc.vector.scalar_tensor_tensor(
                acc_ps[:],
                prob_sb,
                0.5,
                rs_sb,
                op0=mybir.AluOpType.is_ge,
                op1=mybir.AluOpType.mult,
            )

            # ---- X = PY @ TBstack, split in two halves so the PSUM->SBUF
            # fp16 copies pipeline with the mm2 accumulation chain --------
            HW2 = R * H // 2
            x_ps_a = ps.tile([H, HW2], f32)
            x_ps_b = ps.tile([H, HW2], f32)
            x_sb = sb.tile([H, R * H], f16)
            nc.tensor.matmul(
                x_ps_a[:], pytbin[:], tb_sb[:, 0:HW2], start=True, stop=True
            )
            nc.tensor.matmul(
                x_ps_b[:], pytbin[:], tb_sb[:, HW2:R * H], start=True, stop=True
            )
            nc.vector.tensor_copy(x_sb[:, 0:HW2], x_ps_a[:])
            nc.vector.tensor_copy(x_sb[:, HW2:R * H], x_ps_b[:])

            # ---- acc += sum_m TA_m^T @ X_m  (4 accumulating matmuls) ---
            for m in range(R):
                nc.tensor.matmul(
                    acc_ps[:],
                    ta_sb[:, m * H:(m + 1) * H],
                    x_sb[:, m * H:(m + 1) * H],
                    start=False,
                    stop=(m == R - 1),
                )

            # ---- scalar: abs-reduce rows on DVE (PSUM -> SBUF [100,1]),
            # then an in-partition all-reduce on gpsimd, write out -------
            absrow = sb.tile([H, 1], f32)
            nc.vector.tensor_reduce(
                absrow[:],
                acc_ps[:],
                axis=mybir.AxisListType.X,
                op=mybir.AluOpType.add,
                apply_absolute_value=True,
            )
            red = sb.tile([H, 1], f32)
            nc.gpsimd.partition_all_reduce(
                red[:], absrow[:], channels=H, reduce_op=bass_isa.ReduceOp.add
            )
            nc.sync.dma_start(out_d[:], red[0:1, :])

    nc.compile()
    return nc


_STATE = {}


def _get_state():
    if not _STATE:
        _STATE["consts"] = _host_constants()
        _STATE["nc"] = _build_module()
    return _STATE


def _in_maps(prob_map, gt_map):
    st = _get_state()
    rowsum_neg_scaled, ta, tb = st["consts"]
    prob = np.asarray(prob_map, dtype=np.float32).reshape(H, H)
    gt = np.asarray(gt_map, dtype=np.float32).reshape(H, H)
    pk = np.ascontiguousarray(
        np.concatenate([gt.T, prob, rowsum_neg_scaled], axis=1)
    )
    in_map = {"pk": pk, "tb": tb, "ta": ta}
    return [in_map] * NCORES


def _run(prob_map, gt_map, trace=False, **spmd_kwargs):
    from concourse import bass_utils

    st = _get_state()
    in_maps = _in_maps(prob_map, gt_map)
    res = bass_utils.run_bass_kernel_spmd(
        st["nc"], in_maps, core_ids=list(range(NCORES)), trace=trace,
        **spmd_kwargs,
    )
    value = np.float32(res.results[0]["out"][0, 0])
    return value, res


def kernel(prob_map, gt_map):
    value, _ = _run(prob_map, gt_map, trace=False)
    return np.asarray(value, dtype=np.float32)


# revision 22
# speedup vs baseline: 2.1434x; 1.1113x over previous
"""Trainium2 Bass kernel for nn_HausdorffDistance_28406913696124.

Math (reference):
    px = (prob_map[0].ravel() >= 0.5)                 # [N], N = 100*100
    py = (gt_map.ravel()   >= 0.5)                    # [N]
    D[i,j] = euclid dist between grid points i, j     # [N, N] constant!
    loss   = mean_i | px_i * mean_j D[i,j] - (D @ py)_i / N |

Key structure: with pixels i = (r, c), D depends only on the lag pair
(|r-a|, |c-b|), so

    term2[r,c] = (D @ py)[r,c] = sum_{a,b} PY[a,b] * K(|r-a|, |c-b|),
    K(u,v) = sqrt(u^2 + v^2).

K is numerically LOW-RANK: its eigenvalues on the [0,100)^2 lag grid decay
as 8046, -962, -68, -12, -2.9, ...  A rank-4 symmetric eigen-expansion
K ~ sum_m lam_m w_m w_m^T makes term2 separable:

    term2 = sum_m TA_m^T @ PY @ TB_m,   TA_m/TB_m 100x100 symmetric
    Toeplitz tables toep(w_m)*sqrt|lam_m| (sign folded into TA_m).

End-to-end elementwise error of the rank-4 fp16-table pipeline vs the
exact oracle is < 4e-4 (tolerance 2e-2).

Device program per core (ALL work on device, ~15 instructions):
    pytbin = (gtT >= 0.5)                    # DVE, fp16 [100,100]
    X      = PY @ TBstack                    # 1 matmul  [100 x 100 x 400]
    acc    = -px .* rowsum*1e-8              # DVE writes PSUM (term1, exact
                                             #  host-precomputed rowsums)
    acc   += sum_m TA_m^T @ X_m              # 4 accumulating matmuls
    out    = sum |acc|                       # one gpsimd XYZWC abs-reduce
The 1/N^2 is folded into the constant tables (1e-4 per factor side).

Distribution: the whole problem is ~160KB of constants + 2 matmul chains;
a cross-core reduction would cost more in collective latency (~5-15us
floor) than 8-way parallelism saves, so the kernel is replicated on all
8 cores (each computes the identical full scalar, no collectives) and
core 0's output is returned.
"""

import sys

import numpy as np

sys.path.insert(0, "/opt/trn_rl_repo")

H = 100
N = H * H
NCORES = 8
R = 3       # separable rank (validated: ~1e-6 scalar / 4e-4 elementwise err)
S1 = 1e-4   # per-side scale; S1*S1 = 1/N^2 folds the final mean
N_WARM = 0  # PE p-state warm-keeping junk matmuls (no effect in cost model)


def _host_constants():
    """Geometry-only constant tables (input independent)."""
    idx = np.arange(H)
    absdiff = np.abs(idx[:, None] - idx[None, :])  # [100,100] |lag|
    # fp32-exact integer squares -> correctly rounded fp32 sqrt matches the
    # reference's gram-matrix + sqrt construction of D.
    q32 = np.sqrt((idx[:, None] ** 2 + idx[None, :] ** 2).astype(np.float32))

    # Exact per-pixel rowsums of D, accumulated in float64 (term1 path).
    cnt = np.zeros((H, H))  # cnt[r,u] = #{a : |r-a| = u}
    np.add.at(cnt, (idx[:, None], absdiff), 1.0)
    rowsum = cnt @ q32.astype(np.float64) @ cnt.T  # [100,100], ~5.7e5
    rowsum_neg_scaled = (-rowsum * (S1 * S1)).astype(np.float32)

    # Rank-R symmetric eigen-factorization of the lag kernel.
    lam, w = np.linalg.eigh(q32.astype(np.float64))
    order = np.argsort(-np.abs(lam))
    lam, w = lam[order], w[:, order]
    tb = np.zeros((H, R * H), dtype=np.float16)
    ta = np.zeros((H, R * H), dtype=np.float16)
    for m in range(R):
        toep = w[:, m][absdiff] * (np.sqrt(abs(lam[m])) * S1)
        tb[:, m * H:(m + 1) * H] = toep.astype(np.float16)
        ta[:, m * H:(m + 1) * H] = (np.sign(lam[m]) * toep).astype(np.float16)
    return rowsum_neg_scaled, ta, tb


def _build_module():
    import concourse.bacc as bacc
    import concourse.bass as bass
    import concourse.bass_isa as bass_isa
    import concourse.mybir as mybir
    import concourse.tile as tile

    f32 = mybir.dt.float32
    f16 = mybir.dt.float16

    # Bass.__init__ registers four const-AP memsets on the gpsimd/Pool
    # queue; nothing in this kernel reads those const tiles, but the
    # memsets delay the startup all-engine barrier and with it the input
    # DMA descriptor generation by ~0.5us.  Skip them (the const tiles
    # stay allocated; correctness is covered by the numerics tests and
    # the BIR verifier, which already flags the tiles as reader-less).
    orig_memset = bass.BassGpSimd.memset

    def _memset_skip(self, ap, constant):
        return None

    bass.BassGpSimd.memset = _memset_skip
    try:
        nc = bacc.Bacc(
            "TRN2",
            target_bir_lowering=False,
            debug=False,
            enable_asserts=False,
            num_devices=NCORES,
        )
    finally:
        bass.BassGpSimd.memset = orig_memset

    # pk = gtT | prob | rowsum_neg_scaled   ([100, 300] f32)
    pk_d = nc.dram_tensor("pk", [H, 3 * H], f32, kind="ExternalInput")
    tb_d = nc.dram_tensor("tb", [H, R * H], f16, kind="ExternalInput")
    ta_d = nc.dram_tensor("ta", [H, R * H], f16, kind="ExternalInput")
    out_d = nc.dram_tensor("out", [1, 1], f32, kind="ExternalOutput")

    with tile.TileContext(nc) as tc:
        with (
            tc.tile_pool(name="sb", bufs=1) as sb,
            tc.tile_pool(name="ps", bufs=1, space="PSUM") as ps,
        ):
            # ---- loads on three parallel desc-gen paths: pk first on the
            # SP/HWDGE queue (fastest; it gates the binarize -> mm1 chain),
            # tb on the Pool/SWDGE queue, ta second on SP (only needed by
            # the later mm2 chain). ---------------------------------------
            pk_sb = sb.tile([H, 3 * H], f32)
            nc.sync.dma_start(pk_sb[:], pk_d[:])
            tb_sb = sb.tile([H, R * H], f16)
            nc.gpsimd.dma_start(tb_sb[:], tb_d[:])
            ta_sb = sb.tile([H, R * H], f16)
            nc.sync.dma_start(ta_sb[:], ta_d[:])

            gtT_sb = pk_sb[:, 0:H]
            prob_sb = pk_sb[:, H:2 * H]
            rs_sb = pk_sb[:, 2 * H:3 * H]

            # ---- PE p-state warm-keeping: a stream of dependency-free
            # junk matmuls keeps the Tensor engine continuously busy from
            # t~0.5us so the real matmuls run at ramped clock instead of
            # the cold 0.65GHz p-state.  zwarm is zeroed first; the junk
            # results go to a never-read PSUM tile. -----------------------
            zwarm = sb.tile([H, H], f16)
            nc.vector.memset(zwarm[:], 0.0)
            warm_ps = ps.tile([H, H], f32)
            for _ in range(N_WARM):
                nc.tensor.matmul(
                    warm_ps[:], zwarm[:], zwarm[:], start=True, stop=True
                )

            # ---- binarize transposed gt mask (mm1 stationary) -----------
            pytbin = sb.tile([H, H], f16)
            nc.vector.tensor_scalar(
                pytbin[:], gtT_sb, 0.5, None, mybir.AluOpType.is_ge
            )

            # ---- term1 pre-loaded into the mm2 accumulator: ------------
            #      acc = (prob >= 0.5) * (-rowsum * 1e-8)
            acc_ps = ps.tile([H, H], f32)
            nc.vector.scalar_tensor_tensor(
                acc_ps[:],
                prob_sb,
                0.5,
                rs_sb,
                op0=mybir.AluOpType.is_ge,
                op1=mybir.AluOpType.mult,
            )

            # ---- X = PY @ TBstack, split in two halves so the PSUM->SBUF
            # fp16 copies pipeline with the mm2 accumulation chain --------
            HW2 = R * H // 2
            x_ps_a = ps.tile([H, HW2], f32)
            x_ps_b = ps.tile([H, HW2], f32)
            x_sb = sb.tile([H, R * H], f16)
            nc.tensor.matmul(
                x_ps_a[:], pytbin[:], tb_sb[:, 0:HW2], start=True, stop=True
            )
            nc.tensor.matmul(
                x_ps_b[:], pytbin[:], tb_sb[:, HW2:R * H], start=True, stop=True
            )
            nc.vector.tensor_copy(x_sb[:, 0:HW2], x_ps_a[:])
            nc.vector.tensor_copy(x_sb[:, HW2:R * H], x_ps_b[:])

            # ---- acc += sum_m TA_m^T @ X_m  (4 accumulating matmuls) ---
            for m in range(R):
                nc.tensor.matmul(
                    acc_ps[:],
                    ta_sb[:, m * H:(m + 1) * H],
                    x_sb[:, m * H:(m + 1) * H],
                    start=False,
                    stop=(m == R - 1),
                )

            # ---- scalar: abs-reduce rows on DVE (PSUM -> SBUF [100,1]),
            # then an in-partition all-reduce on gpsimd, write out -------
            absrow = sb.tile([H, 1], f32)
            nc.vector.tensor_reduce(
                absrow[:],
                acc_ps[:],
                axis=mybir.AxisListType.X,
                op=mybir.AluOpType.add,
                apply_absolute_value=True,
            )
            red = sb.tile([H, 1], f32)
            nc.gpsimd.partition_all_reduce(
                red[:], absrow[:], channels=H, reduce_op=bass_isa.ReduceOp.add
            )
            nc.sync.dma_start(out_d[:], red[0:1, :])

    nc.compile()
    return nc


_STATE = {}


def _get_state():
    if not _STATE:
        _STATE["consts"] = _host_constants()
        _STATE["nc"] = _build_module()
    return _STATE


def _in_maps(prob_map, gt_map):
    st = _get_state()
    rowsum_neg_scaled, ta, tb = st["consts"]
    prob = np.asarray(prob_map, dtype=np.float32).reshape(H, H)
    gt = np.asarray(gt_map, dtype=np.float32).reshape(H, H)
    pk = np.ascontiguousarray(
        np.concatenate([gt.T, prob, rowsum_neg_scaled], axis=1)
    )
    in_map = {"pk": pk, "tb": tb, "ta": ta}
    return [in_map] * NCORES


def _run(prob_map, gt_map, trace=False, **spmd_kwargs):
    from concourse import bass_utils

    st = _get_state()
    in_maps = _in_maps(prob_map, gt_map)
    res = bass_utils.run_bass_kernel_spmd(
        st["nc"], in_maps, core_ids=list(range(NCORES)), trace=trace,
        **spmd_kwargs,
    )
    value = np.float32(np.asarray(res.results[0]["out"]).reshape(-1)[0])
    return value, res


def kernel(prob_map, gt_map):
    value, _ = _run(prob_map, gt_map, trace=False)
    return np.asarray(value, dtype=np.float32)


# revision 23
# speedup vs baseline: 2.1629x; 1.0091x over previous
"""Trainium2 Bass kernel for nn_HausdorffDistance_28406913696124.

Math (reference):
    px = (prob_map[0].ravel() >= 0.5)                 # [N], N = 100*100
    py = (gt_map.ravel()   >= 0.5)                    # [N]
    D[i,j] = euclid dist between grid points i, j     # [N, N] constant!
    loss   = mean_i | px_i * mean_j D[i,j] - (D @ py)_i / N |

Key structure: with pixels i = (r, c), D depends only on the lag pair
(|r-a|, |c-b|), so

    term2[r,c] = (D @ py)[r,c] = sum_{a,b} PY[a,b] * K(|r-a|, |c-b|),
    K(u,v) = sqrt(u^2 + v^2).

K is numerically LOW-RANK: its eigenvalues on the [0,100)^2 lag grid decay
as 8046, -962, -68, -12, -2.9, ...  A rank-4 symmetric eigen-expansion
K ~ sum_m lam_m w_m w_m^T makes term2 separable:

    term2 = sum_m TA_m^T @ PY @ TB_m,   TA_m/TB_m 100x100 symmetric
    Toeplitz tables toep(w_m)*sqrt|lam_m| (sign folded into TA_m).

End-to-end elementwise error of the rank-4 fp16-table pipeline vs the
exact oracle is < 4e-4 (tolerance 2e-2).

Device program per core (ALL work on device, ~15 instructions):
    pytbin = (gtT >= 0.5)                    # DVE, fp16 [100,100]
    X      = PY @ TBstack                    # 1 matmul  [100 x 100 x 400]
    acc    = -px .* rowsum*1e-8              # DVE writes PSUM (term1, exact
                                             #  host-precomputed rowsums)
    acc   += sum_m TA_m^T @ X_m              # 4 accumulating matmuls
    out    = sum |acc|                       # one gpsimd XYZWC abs-reduce
The 1/N^2 is folded into the constant tables (1e-4 per factor side).

Distribution: the whole problem is ~160KB of constants + 2 matmul chains;
a cross-core reduction would cost more in collective latency (~5-15us
floor) than 8-way parallelism saves, so the kernel is replicated on all
8 cores (each computes the identical full scalar, no collectives) and
core 0's output is returned.
"""

import sys

import numpy as np

sys.path.insert(0, "/opt/trn_rl_repo")

H = 100
N = H * H
NCORES = 8
R = 3       # separable rank (validated: ~1e-6 scalar / 4e-4 elementwise err)
S1 = 1e-4   # per-side scale; S1*S1 = 1/N^2 folds the final mean
N_WARM = 0  # PE p-state warm-keeping junk matmuls (no effect in cost model)


def _host_constants():
    """Geometry-only constant tables (input independent)."""
    idx = np.arange(H)
    absdiff = np.abs(idx[:, None] - idx[None, :])  # [100,100] |lag|
    # fp32-exact integer squares -> correctly rounded fp32 sqrt matches the
    # reference's gram-matrix + sqrt construction of D.
    q32 = np.sqrt((idx[:, None] ** 2 + idx[None, :] ** 2).astype(np.float32))

    # Exact per-pixel rowsums of D, accumulated in float64 (term1 path).
    cnt = np.zeros((H, H))  # cnt[r,u] = #{a : |r-a| = u}
    np.add.at(cnt, (idx[:, None], absdiff), 1.0)
    rowsum = cnt @ q32.astype(np.float64) @ cnt.T  # [100,100], ~5.7e5
    rowsum_neg_scaled = (-rowsum * (S1 * S1)).astype(np.float32)

    # Rank-R symmetric eigen-factorization of the lag kernel.
    lam, w = np.linalg.eigh(q32.astype(np.float64))
    order = np.argsort(-np.abs(lam))
    lam, w = lam[order], w[:, order]
    tb = np.zeros((H, R * H), dtype=np.float16)
    ta = np.zeros((H, R * H), dtype=np.float16)
    for m in range(R):
        toep = w[:, m][absdiff] * (np.sqrt(abs(lam[m])) * S1)
        tb[:, m * H:(m + 1) * H] = toep.astype(np.float16)
        ta[:, m * H:(m + 1) * H] = (np.sign(lam[m]) * toep).astype(np.float16)
    return rowsum_neg_scaled, ta, tb


def _build_module():
    import concourse.bacc as bacc
    import concourse.bass as bass
    import concourse.bass_isa as bass_isa
    import concourse.mybir as mybir
    import concourse.tile as tile

    f32 = mybir.dt.float32
    f16 = mybir.dt.float16

    # Bass.__init__ registers four const-AP memsets on the gpsimd/Pool
    # queue; nothing in this kernel reads those const tiles, but the
    # memsets delay the startup all-engine barrier and with it the input
    # DMA descriptor generation by ~0.5us.  Skip them (the const tiles
    # stay allocated; correctness is covered by the numerics tests and
    # the BIR verifier, which already flags the tiles as reader-less).
    orig_memset = bass.BassGpSimd.memset

    def _memset_skip(self, ap, constant):
        return None

    bass.BassGpSimd.memset = _memset_skip
    try:
        nc = bacc.Bacc(
            "TRN2",
            target_bir_lowering=False,
            debug=False,
            enable_asserts=False,
            num_devices=NCORES,
        )
    finally:
        bass.BassGpSimd.memset = orig_memset

    # pk = gtT | prob | rowsum_neg_scaled   ([100, 300] f32)
    pk_d = nc.dram_tensor("pk", [H, 3 * H], f32, kind="ExternalInput")
    tb_d = nc.dram_tensor("tb", [H, R * H], f16, kind="ExternalInput")
    ta_d = nc.dram_tensor("ta", [H, R * H], f16, kind="ExternalInput")
    out_d = nc.dram_tensor("out", [1, 1], f32, kind="ExternalOutput")

    with tile.TileContext(nc) as tc:
        with (
            tc.tile_pool(name="sb", bufs=1) as sb,
            tc.tile_pool(name="ps", bufs=1, space="PSUM") as ps,
        ):
            # ---- loads on three parallel desc-gen paths: pk first on the
            # SP/HWDGE queue (fastest; it gates the binarize -> mm1 chain),
            # tb on the Pool/SWDGE queue, ta second on SP (only needed by
            # the later mm2 chain). ---------------------------------------
            pk_sb = sb.tile([H, 3 * H], f32)
            nc.sync.dma_start(pk_sb[:], pk_d[:])
            tb_sb = sb.tile([H, R * H], f16)
            nc.gpsimd.dma_start(tb_sb[:], tb_d[:])
            ta_sb = sb.tile([H, R * H], f16)
            nc.sync.dma_start(ta_sb[:], ta_d[:])

            gtT_sb = pk_sb[:, 0:H]
            prob_sb = pk_sb[:, H:2 * H]
            rs_sb = pk_sb[:, 2 * H:3 * H]

            # ---- PE p-state warm-keeping: a stream of dependency-free
            # junk matmuls keeps the Tensor engine continuously busy from
            # t~0.5us so the real matmuls run at ramped clock instead of
            # the cold 0.65GHz p-state.  zwarm is zeroed first; the junk
            # results go to a never-read PSUM tile. -----------------------
            zwarm = sb.tile([H, H], f16)
            nc.vector.memset(zwarm[:], 0.0)
            warm_ps = ps.tile([H, H], f32)
            for _ in range(N_WARM):
                nc.tensor.matmul(
                    warm_ps[:], zwarm[:], zwarm[:], start=True, stop=True
                )

            # ---- binarize transposed gt mask (mm1 stationary) -----------
            pytbin = sb.tile([H, H], f16)
            nc.vector.tensor_scalar(
                pytbin[:], gtT_sb, 0.5, None, mybir.AluOpType.is_ge
            )

            # ---- term1 pre-loaded into the mm2 accumulator: ------------
            #      acc = (prob >= 0.5) * (-rowsum * 1e-8)
            acc_ps = ps.tile([H, H], f32)
            nc.vector.scalar_tensor_tensor(
                acc_ps[:],
                prob_sb,
                0.5,
                rs_sb,
                op0=mybir.AluOpType.is_ge,
                op1=mybir.AluOpType.mult,
            )

            # ---- X = PY @ TBstack, split in two halves so the PSUM->SBUF
            # fp16 copies pipeline with the mm2 accumulation chain --------
            HW2 = R * H // 2
            x_ps_a = ps.tile([H, HW2], f32)
            x_ps_b = ps.tile([H, HW2], f32)
            x_sb = sb.tile([H, R * H], f16)
            nc.tensor.matmul(
                x_ps_a[:], pytbin[:], tb_sb[:, 0:HW2], start=True, stop=True
            )
            nc.tensor.matmul(
                x_ps_b[:], pytbin[:], tb_sb[:, HW2:R * H], start=True, stop=True
            )
            # first half on DVE, second half in parallel on the (idle)
            # Activation engine; both read PSUM and downcast to fp16
            nc.vector.tensor_copy(x_sb[:, 0:HW2], x_ps_a[:])
            nc.scalar.copy(x_sb[:, HW2:R * H], x_ps_b[:])

            # ---- acc += sum_m TA_m^T @ X_m  (accumulating matmuls; m=1
            # spans both copies, so it goes last) -------------------------
            order = [0, 2, 1] if R == 3 else list(range(R))
            for i, m in enumerate(order):
                nc.tensor.matmul(
                    acc_ps[:],
                    ta_sb[:, m * H:(m + 1) * H],
                    x_sb[:, m * H:(m + 1) * H],
                    start=False,
                    stop=(i == R - 1),
                )

            # ---- scalar: abs-reduce rows on DVE (PSUM -> SBUF [100,1]),
            # then an in-partition all-reduce on gpsimd, write out -------
            absrow = sb.tile([H, 1], f32)
            nc.vector.tensor_reduce(
                absrow[:],
                acc_ps[:],
                axis=mybir.AxisListType.X,
                op=mybir.AluOpType.add,
                apply_absolute_value=True,
            )
            red = sb.tile([H, 1], f32)
            nc.gpsimd.partition_all_reduce(
                red[:], absrow[:], channels=H, reduce_op=bass_isa.ReduceOp.add
            )
            nc.sync.dma_start(out_d[:], red[0:1, :])

    nc.compile()
    return nc


_STATE = {}


def _get_state():
    if not _STATE:
        _STATE["consts"] = _host_constants()
        _STATE["nc"] = _build_module()
    return _STATE


def _in_maps(prob_map, gt_map):
    st = _get_state()
    rowsum_neg_scaled, ta, tb = st["consts"]
    prob = np.asarray(prob_map, dtype=np.float32).reshape(H, H)
    gt = np.asarray(gt_map, dtype=np.float32).reshape(H, H)
    pk = np.ascontiguousarray(
        np.concatenate([gt.T, prob, rowsum_neg_scaled], axis=1)
    )
    in_map = {"pk": pk, "tb": tb, "ta": ta}
    return [in_map] * NCORES


def _run(prob_map, gt_map, trace=False, **spmd_kwargs):
    from concourse import bass_utils

    st = _get_state()
    in_maps = _in_maps(prob_map, gt_map)
    res = bass_utils.run_bass_kernel_spmd(
        st["nc"], in_maps, core_ids=list(range(NCORES)), trace=trace,
        **spmd_kwargs,
    )
    value = np.float32(np.asarray(res.results[0]["out"]).reshape(-1)[0])
    return value, res


def kernel(prob_map, gt_map):
    value, _ = _run(prob_map, gt_map, trace=False)
    return np.asarray(value, dtype=np.float32)


# revision 26
# speedup vs baseline: 2.2410x; 1.0361x over previous
"""Trainium2 Bass kernel for nn_HausdorffDistance_28406913696124.

Math (reference):
    px = (prob_map[0].ravel() >= 0.5)                 # [N], N = 100*100
    py = (gt_map.ravel()   >= 0.5)                    # [N]
    D[i,j] = euclid dist between grid points i, j     # [N, N] constant!
    loss   = mean_i | px_i * mean_j D[i,j] - (D @ py)_i / N |

Key structure: with pixels i = (r, c), D depends only on the lag pair
(|r-a|, |c-b|), so

    term2[r,c] = (D @ py)[r,c] = sum_{a,b} PY[a,b] * K(|r-a|, |c-b|),
    K(u,v) = sqrt(u^2 + v^2).

K is numerically LOW-RANK: its eigenvalues on the [0,100)^2 lag grid decay
as 8046, -962, -68, -12, -2.9, ...  A rank-4 symmetric eigen-expansion
K ~ sum_m lam_m w_m w_m^T makes term2 separable:

    term2 = sum_m TA_m^T @ PY @ TB_m,   TA_m/TB_m 100x100 symmetric
    Toeplitz tables toep(w_m)*sqrt|lam_m| (sign folded into TA_m).

End-to-end elementwise error of the rank-4 fp16-table pipeline vs the
exact oracle is < 4e-4 (tolerance 2e-2).

Device program per core (ALL work on device, ~15 instructions):
    pytbin = (gtT >= 0.5)                    # DVE, fp16 [100,100]
    X      = PY @ TBstack                    # 1 matmul  [100 x 100 x 400]
    acc    = -px .* rowsum*1e-8              # DVE writes PSUM (term1, exact
                                             #  host-precomputed rowsums)
    acc   += sum_m TA_m^T @ X_m              # 4 accumulating matmuls
    out    = sum |acc|                       # one gpsimd XYZWC abs-reduce
The 1/N^2 is folded into the constant tables (1e-4 per factor side).

Distribution: the whole problem is ~160KB of constants + 2 matmul chains;
a cross-core reduction would cost more in collective latency (~5-15us
floor) than 8-way parallelism saves, so the kernel is replicated on all
8 cores (each computes the identical full scalar, no collectives) and
core 0's output is returned.
"""

import sys

import numpy as np

sys.path.insert(0, "/opt/trn_rl_repo")

H = 100
N = H * H
NCORES = 8
R = 3       # separable rank (validated: ~1e-6 scalar / 4e-4 elementwise err)
S1 = 1e-4   # per-side scale; S1*S1 = 1/N^2 folds the final mean


def _host_constants():
    """Geometry-only constant tables (input independent)."""
    idx = np.arange(H)
    absdiff = np.abs(idx[:, None] - idx[None, :])  # [100,100] |lag|
    # fp32-exact integer squares -> correctly rounded fp32 sqrt matches the
    # reference's gram-matrix + sqrt construction of D.
    q32 = np.sqrt((idx[:, None] ** 2 + idx[None, :] ** 2).astype(np.float32))

    # Exact per-pixel rowsums of D, accumulated in float64 (term1 path).
    cnt = np.zeros((H, H))  # cnt[r,u] = #{a : |r-a| = u}
    np.add.at(cnt, (idx[:, None], absdiff), 1.0)
    rowsum = cnt @ q32.astype(np.float64) @ cnt.T  # [100,100], ~5.7e5
    rowsum_neg_scaled = (-rowsum * (S1 * S1)).astype(np.float32)

    # Rank-R symmetric eigen-factorization of the lag kernel.
    lam, w = np.linalg.eigh(q32.astype(np.float64))
    order = np.argsort(-np.abs(lam))
    lam, w = lam[order], w[:, order]
    tb = np.zeros((H, R * H), dtype=np.float16)
    ta = np.zeros((H, R * H), dtype=np.float16)
    for m in range(R):
        toep = w[:, m][absdiff] * (np.sqrt(abs(lam[m])) * S1)
        tb[:, m * H:(m + 1) * H] = toep.astype(np.float16)
        ta[:, m * H:(m + 1) * H] = (np.sign(lam[m]) * toep).astype(np.float16)
    return rowsum_neg_scaled, ta, tb


def _build_module():
    import concourse.bacc as bacc
    import concourse.bass as bass
    import concourse.bass_isa as bass_isa
    import concourse.mybir as mybir
    import concourse.tile as tile

    f32 = mybir.dt.float32
    f16 = mybir.dt.float16

    # Bass.__init__ registers four const-AP memsets on the gpsimd/Pool
    # queue; nothing in this kernel reads those const tiles, but the
    # memsets delay the startup all-engine barrier and with it the input
    # DMA descriptor generation by ~0.5us.  Skip them (the const tiles
    # stay allocated; correctness is covered by the numerics tests and
    # the BIR verifier, which already flags the tiles as reader-less).
    orig_memset = bass.BassGpSimd.memset

    def _memset_skip(self, ap, constant):
        return None

    bass.BassGpSimd.memset = _memset_skip
    try:
        nc = bacc.Bacc(
            "TRN2",
            target_bir_lowering=False,
            debug=False,
            enable_asserts=False,
            num_devices=NCORES,
        )
    finally:
        bass.BassGpSimd.memset = orig_memset

    # TileContext's exit epilogue is drain -> barrier -> semaphore clears
    # -> barrier.  The second barrier only re-synchronizes engines after
    # the clears; each engine's queue must drain before the NEFF completes
    # anyway, so it adds ~0.26us of pure shutdown latency.  Skip it.
    orig_dab = tile.TileContext._drain_and_barrier

    def _drain_and_barrier_single(self, tick_clock, wait_clock):
        drain_inst = self.nc.sync.drain()
        wait_clock.add_sem_waits(
            drain_inst.ins, tile.ScopedClock({None: tick_clock.global_clock})
        )
        self.nc.all_engine_barrier()
        popped = self.nc._tile_sem_poison_stack.pop()
        assert popped is self._sem_poison
        self.nc.clear_and_free_semaphores(list(self.sems.allocated().values()))

    tile.TileContext._drain_and_barrier = _drain_and_barrier_single

    # pk = gtT | prob | rowsum_neg_scaled   ([100, 300] f32)
    pk_d = nc.dram_tensor("pk", [H, 3 * H], f32, kind="ExternalInput")
    tb_d = nc.dram_tensor("tb", [H, R * H], f16, kind="ExternalInput")
    ta_d = nc.dram_tensor("ta", [H, R * H], f16, kind="ExternalInput")
    out_d = nc.dram_tensor("out", [1, 1], f32, kind="ExternalOutput")

    with tile.TileContext(nc) as tc:
        with (
            tc.tile_pool(name="sb", bufs=1) as sb,
            tc.tile_pool(name="ps", bufs=1, space="PSUM") as ps,
        ):
            # ---- loads on three parallel desc-gen paths: pk first on the
            # SP/HWDGE queue (fastest; it gates the binarize -> mm1 chain),
            # tb on the Pool/SWDGE queue, ta second on SP (only needed by
            # the later mm2 chain). ---------------------------------------
            pk_sb = sb.tile([H, 3 * H], f32)
            nc.sync.dma_start(pk_sb[:], pk_d[:])
            tb_sb = sb.tile([H, R * H], f16)
            nc.gpsimd.dma_start(tb_sb[:], tb_d[:])
            ta_sb = sb.tile([H, R * H], f16)
            nc.sync.dma_start(ta_sb[:], ta_d[:])

            gtT_sb = pk_sb[:, 0:H]
            prob_sb = pk_sb[:, H:2 * H]
            rs_sb = pk_sb[:, 2 * H:3 * H]

            # ---- binarize transposed gt mask (mm1 stationary) -----------
            pytbin = sb.tile([H, H], f16)
            nc.vector.tensor_scalar(
                pytbin[:], gtT_sb, 0.5, None, mybir.AluOpType.is_ge
            )

            # ---- term1 pre-loaded into the mm2 accumulator: ------------
            #      acc = (prob >= 0.5) * (-rowsum * 1e-8)
            acc_ps = ps.tile([H, H], f32)
            nc.vector.scalar_tensor_tensor(
                acc_ps[:],
                prob_sb,
                0.5,
                rs_sb,
                op0=mybir.AluOpType.is_ge,
                op1=mybir.AluOpType.mult,
            )

            # ---- X = PY @ TBstack, split in two halves so the PSUM->SBUF
            # fp16 copies pipeline with the mm2 accumulation chain --------
            HW2 = R * H // 2
            x_ps_a = ps.tile([H, HW2], f32)
            x_ps_b = ps.tile([H, HW2], f32)
            x_sb = sb.tile([H, R * H], f16)
            nc.tensor.matmul(
                x_ps_a[:], pytbin[:], tb_sb[:, 0:HW2], start=True, stop=True
            )
            nc.tensor.matmul(
                x_ps_b[:], pytbin[:], tb_sb[:, HW2:R * H], start=True, stop=True
            )
            # first half on DVE, second half in parallel on the (idle)
            # Activation engine; both read PSUM and downcast to fp16
            nc.vector.tensor_copy(x_sb[:, 0:HW2], x_ps_a[:])
            nc.scalar.copy(x_sb[:, HW2:R * H], x_ps_b[:])

            # ---- acc += sum_m TA_m^T @ X_m  (accumulating matmuls; m=1
            # spans both copies, so it goes last) -------------------------
            order = [0, 2, 1] if R == 3 else list(range(R))
            for i, m in enumerate(order):
                nc.tensor.matmul(
                    acc_ps[:],
                    ta_sb[:, m * H:(m + 1) * H],
                    x_sb[:, m * H:(m + 1) * H],
                    start=False,
                    stop=(i == R - 1),
                )

            # ---- scalar: abs-reduce rows on DVE (PSUM -> SBUF [100,1]),
            # then an in-partition all-reduce on gpsimd, write out -------
            absrow = sb.tile([H, 1], f32)
            nc.vector.tensor_reduce(
                absrow[:],
                acc_ps[:],
                axis=mybir.AxisListType.X,
                op=mybir.AluOpType.add,
                apply_absolute_value=True,
            )
            red = sb.tile([H, 1], f32)
            nc.gpsimd.partition_all_reduce(
                red[:], absrow[:], channels=H, reduce_op=bass_isa.ReduceOp.add
            )
            nc.sync.dma_start(out_d[:], red[0:1, :])

    tile.TileContext._drain_and_barrier = orig_dab
    nc.compile()
    return nc


_STATE = {}


def _get_state():
    if not _STATE:
        _STATE["consts"] = _host_constants()
        _STATE["nc"] = _build_module()
    return _STATE


def _in_maps(prob_map, gt_map):
    st = _get_state()
    rowsum_neg_scaled, ta, tb = st["consts"]
    prob = np.asarray(prob_map, dtype=np.float32).reshape(H, H)
    gt = np.asarray(gt_map, dtype=np.float32).reshape(H, H)
    pk = np.ascontiguousarray(
        np.concatenate([gt.T, prob, rowsum_neg_scaled], axis=1)
    )
    in_map = {"pk": pk, "tb": tb, "ta": ta}
    return [in_map] * NCORES


def _run(prob_map, gt_map, trace=False, **spmd_kwargs):
    from concourse import bass_utils

    st = _get_state()
    in_maps = _in_maps(prob_map, gt_map)
    res = bass_utils.run_bass_kernel_spmd(
        st["nc"], in_maps, core_ids=list(range(NCORES)), trace=trace,
        **spmd_kwargs,
    )
    value = np.float32(np.asarray(res.results[0]["out"]).reshape(-1)[0])
    return value, res


def kernel(prob_map, gt_map):
    value, _ = _run(prob_map, gt_map, trace=False)
    return np.asarray(value, dtype=np.float32)


# revision 28
# speedup vs baseline: 3.0904x; 1.3791x over previous
"""Trainium2 Bass kernel for nn_HausdorffDistance_28406913696124.

Math (reference):
    px = (prob_map[0].ravel() >= 0.5)                 # [N], N = 100*100
    py = (gt_map.ravel()   >= 0.5)                    # [N]
    D[i,j] = euclid dist between grid points i, j     # [N, N] constant!
    loss   = mean_i | px_i * mean_j D[i,j] - (D @ py)_i / N |

Key structure: with pixels i = (r, c), D depends only on the lag pair
(|r-a|, |c-b|), so

    term2[r,c] = (D @ py)[r,c] = sum_{a,b} PY[a,b] * K(|r-a|, |c-b|),
    K(u,v) = sqrt(u^2 + v^2).

K is numerically LOW-RANK: its eigenvalues on the [0,100)^2 lag grid decay
as 8046, -962, -68, -12, -2.9, ...  A rank-4 symmetric eigen-expansion
K ~ sum_m lam_m w_m w_m^T makes term2 separable:

    term2 = sum_m TA_m^T @ PY @ TB_m,   TA_m/TB_m 100x100 symmetric
    Toeplitz tables toep(w_m)*sqrt|lam_m| (sign folded into TA_m).

End-to-end elementwise error of the rank-4 fp16-table pipeline vs the
exact oracle is < 4e-4 (tolerance 2e-2).

Device program per core (ALL work on device, ~15 instructions):
    pytbin = (gtT >= 0.5)                    # DVE, fp16 [100,100]
    X      = PY @ TBstack                    # 1 matmul  [100 x 100 x 400]
    acc    = -px .* rowsum*1e-8              # DVE writes PSUM (term1, exact
                                             #  host-precomputed rowsums)
    acc   += sum_m TA_m^T @ X_m              # 4 accumulating matmuls
    out    = sum |acc|                       # one gpsimd XYZWC abs-reduce
The 1/N^2 is folded into the constant tables (1e-4 per factor side).

Distribution: the whole problem is ~160KB of constants + 2 matmul chains;
a cross-core reduction would cost more in collective latency (~5-15us
floor) than 8-way parallelism saves, so the kernel is replicated on all
8 cores (each computes the identical full scalar, no collectives) and
core 0's output is returned.
"""

import sys

import numpy as np

sys.path.insert(0, "/opt/trn_rl_repo")

H = 100
N = H * H
NCORES = 8
R = 3       # separable rank (validated: ~1e-6 scalar / 4e-4 elementwise err)
S1 = 1e-4   # per-side scale; S1*S1 = 1/N^2 folds the final mean


def _host_constants():
    """Geometry-only constant tables (input independent)."""
    idx = np.arange(H)
    absdiff = np.abs(idx[:, None] - idx[None, :])  # [100,100] |lag|
    # fp32-exact integer squares -> correctly rounded fp32 sqrt matches the
    # reference's gram-matrix + sqrt construction of D.
    q32 = np.sqrt((idx[:, None] ** 2 + idx[None, :] ** 2).astype(np.float32))

    # Exact per-pixel rowsums of D, accumulated in float64 (term1 path).
    cnt = np.zeros((H, H))  # cnt[r,u] = #{a : |r-a| = u}
    np.add.at(cnt, (idx[:, None], absdiff), 1.0)
    rowsum = cnt @ q32.astype(np.float64) @ cnt.T  # [100,100], ~5.7e5
    rowsum_neg_scaled = (-rowsum * (S1 * S1)).astype(np.float32)

    # Rank-R symmetric eigen-factorization of the lag kernel.
    lam, w = np.linalg.eigh(q32.astype(np.float64))
    order = np.argsort(-np.abs(lam))
    lam, w = lam[order], w[:, order]
    tb = np.zeros((H, R * H), dtype=np.float16)
    ta = np.zeros((H, R * H), dtype=np.float16)
    for m in range(R):
        toep = w[:, m][absdiff] * (np.sqrt(abs(lam[m])) * S1)
        tb[:, m * H:(m + 1) * H] = toep.astype(np.float16)
        ta[:, m * H:(m + 1) * H] = (np.sign(lam[m]) * toep).astype(np.float16)
    return rowsum_neg_scaled, ta, tb


def _build_module():
    import concourse.bacc as bacc
    import concourse.bass as bass
    import concourse.bass_isa as bass_isa
    import concourse.mybir as mybir
    import concourse.tile as tile

    f32 = mybir.dt.float32
    f16 = mybir.dt.float16

    # Bass.__init__ registers four const-AP memsets on the gpsimd/Pool
    # queue; nothing in this kernel reads those const tiles, but the
    # memsets delay the startup all-engine barrier and with it the input
    # DMA descriptor generation by ~0.5us.  Skip them (the const tiles
    # stay allocated; correctness is covered by the numerics tests and
    # the BIR verifier, which already flags the tiles as reader-less).
    orig_memset = bass.BassGpSimd.memset

    def _memset_skip(self, ap, constant):
        return None

    bass.BassGpSimd.memset = _memset_skip
    try:
        nc = bacc.Bacc(
            "TRN2",
            target_bir_lowering=False,
            debug=False,
            enable_asserts=False,
            num_devices=NCORES,
        )
    finally:
        bass.BassGpSimd.memset = orig_memset

    # TileContext's exit epilogue is drain -> barrier -> semaphore clears
    # -> barrier.  The second barrier only re-synchronizes engines after
    # the clears; each engine's queue must drain before the NEFF completes
    # anyway, so it adds ~0.26us of pure shutdown latency.  Skip it.
    orig_dab = tile.TileContext._drain_and_barrier

    def _drain_and_barrier_single(self, tick_clock, wait_clock):
        drain_inst = self.nc.sync.drain()
        wait_clock.add_sem_waits(
            drain_inst.ins, tile.ScopedClock({None: tick_clock.global_clock})
        )
        self.nc.all_engine_barrier()
        popped = self.nc._tile_sem_poison_stack.pop()
        assert popped is self._sem_poison
        self.nc.clear_and_free_semaphores(list(self.sems.allocated().values()))

    tile.TileContext._drain_and_barrier = _drain_and_barrier_single

    # pk = gtT | prob | rowsum_neg_scaled   ([100, 300] f32)
    pk_d = nc.dram_tensor("pk", [H, 3 * H], f32, kind="ExternalInput")
    tb_d = nc.dram_tensor("tb", [H, R * H], f16, kind="ExternalInput")
    ta_d = nc.dram_tensor("ta", [H, R * H], f16, kind="ExternalInput")
    out_d = nc.dram_tensor("out", [1, 1], f32, kind="ExternalOutput")

    with tile.TileContext(nc) as tc:
        with (
            tc.tile_pool(name="sb", bufs=1) as sb,
            tc.tile_pool(name="ps", bufs=1, space="PSUM") as ps,
        ):
            # ---- loads on three parallel desc-gen paths: pk first on the
            # SP/HWDGE queue (fastest; it gates the binarize -> mm1 chain),
            # tb on the Pool/SWDGE queue, ta second on SP (only needed by
            # the later mm2 chain). ---------------------------------------
            pk_sb = sb.tile([H, 3 * H], f32)
            nc.sync.dma_start(pk_sb[:], pk_d[:])
            tb_sb = sb.tile([H, R * H], f16)
            nc.gpsimd.dma_start(tb_sb[:], tb_d[:])
            ta_sb = sb.tile([H, R * H], f16)
            nc.sync.dma_start(ta_sb[:], ta_d[:])

            gtT_sb = pk_sb[:, 0:H]
            prob_sb = pk_sb[:, H:2 * H]
            rs_sb = pk_sb[:, 2 * H:3 * H]

            # ---- binarize transposed gt mask (mm1 stationary) -----------
            pytbin = sb.tile([H, H], f16)
            nc.vector.tensor_scalar(
                pytbin[:], gtT_sb, 0.5, None, mybir.AluOpType.is_ge
            )

            # ---- term1 pre-loaded into the mm2 accumulator: ------------
            #      acc = (prob >= 0.5) * (-rowsum * 1e-8)
            acc_ps = ps.tile([H, H], f32)
            nc.vector.scalar_tensor_tensor(
                acc_ps[:],
                prob_sb,
                0.5,
                rs_sb,
                op0=mybir.AluOpType.is_ge,
                op1=mybir.AluOpType.mult,
            )

            # ---- X = PY @ TBstack, split in two halves so the PSUM->SBUF
            # fp16 copies pipeline with the mm2 accumulation chain --------
            HW2 = R * H // 2
            x_ps_a = ps.tile([H, HW2], f32)
            x_ps_b = ps.tile([H, HW2], f32)
            x_sb = sb.tile([H, R * H], f16)
            nc.tensor.matmul(
                x_ps_a[:], pytbin[:], tb_sb[:, 0:HW2], start=True, stop=True
            )
            nc.tensor.matmul(
                x_ps_b[:], pytbin[:], tb_sb[:, HW2:R * H], start=True, stop=True
            )
            # first half on DVE, second half in parallel on the (idle)
            # Activation engine; both read PSUM and downcast to fp16
            nc.vector.tensor_copy(x_sb[:, 0:HW2], x_ps_a[:])
            nc.scalar.copy(x_sb[:, HW2:R * H], x_ps_b[:])

            # ---- acc += sum_m TA_m^T @ X_m  (accumulating matmuls; m=1
            # spans both copies, so it goes last) -------------------------
            order = [0, 2, 1] if R == 3 else list(range(R))
            for i, m in enumerate(order):
                nc.tensor.matmul(
                    acc_ps[:],
                    ta_sb[:, m * H:(m + 1) * H],
                    x_sb[:, m * H:(m + 1) * H],
                    start=False,
                    stop=(i == R - 1),
                )

            # ---- scalar: abs-reduce rows on DVE (PSUM -> SBUF [100,1]),
            # then an in-partition all-reduce on gpsimd, write out -------
            absrow = sb.tile([H, 1], f32)
            nc.vector.tensor_reduce(
                absrow[:],
                acc_ps[:],
                axis=mybir.AxisListType.X,
                op=mybir.AluOpType.add,
                apply_absolute_value=True,
            )
            red = sb.tile([H, 1], f32)
            nc.gpsimd.partition_all_reduce(
                red[:], absrow[:], channels=H, reduce_op=bass_isa.ReduceOp.add
            )
            # ---- output store via GPSIMD register passthrough: load the
            # 4 result bytes into a Pool-sequencer GPR and store them to
            # DRAM directly -- skips the whole DMA fixed-latency chain
            # (desc-gen + DGE delay + completion-semaphore propagation). --
            i32 = mybir.dt.int32
            with nc.gpsimd.register("out_val") as out_reg:
                nc.gpsimd.reg_load(out_reg, red[0:1, 0:1].bitcast(i32))
                nc.gpsimd.reg_save(out_d[0:1, 0:1].bitcast(i32), out_reg)

    tile.TileContext._drain_and_barrier = orig_dab
    nc.compile()
    return nc


_STATE = {}


def _get_state():
    if not _STATE:
        _STATE["consts"] = _host_constants()
        _STATE["nc"] = _build_module()
    return _STATE


def _in_maps(prob_map, gt_map):
    st = _get_state()
    rowsum_neg_scaled, ta, tb = st["consts"]
    prob = np.asarray(prob_map, dtype=np.float32).reshape(H, H)
    gt = np.asarray(gt_map, dtype=np.float32).reshape(H, H)
    pk = np.ascontiguousarray(
        np.concatenate([gt.T, prob, rowsum_neg_scaled], axis=1)
    )
    in_map = {"pk": pk, "tb": tb, "ta": ta}
    return [in_map] * NCORES


def _run(prob_map, gt_map, trace=False, **spmd_kwargs):
    from concourse import bass_utils

    st = _get_state()
    in_maps = _in_maps(prob_map, gt_map)
    res = bass_utils.run_bass_kernel_spmd(
        st["nc"], in_maps, core_ids=list(range(NCORES)), trace=trace,
        **spmd_kwargs,
    )
    value = np.float32(np.asarray(res.results[0]["out"]).reshape(-1)[0])
    return value, res


def kernel(prob_map, gt_map):
    value, _ = _run(prob_map, gt_map, trace=False)
    return np.asarray(value, dtype=np.float32)
